# revision 19
# baseline (speedup 1.0000x reference)
"""Trainium2 Bass kernel for nn_AttnBlock3d (BatchNorm3d + single-head
self-attention over N=4096 voxels + residual), distributed over 8 NeuronCores.

Sharding: data-parallel over batch (2) x query-quarters (4). Each core
receives its batch's activations (xb), the other batch (xo, stats only),
its query slice (xq, fp32 for the residual), and the weights; it returns
its (C, 1024) output slice. Host assembles the full (B, C, D, H, W) output.

Math notes:
 - BatchNorm folds into the projection weights: with a = gamma*rsqrt(var+eps)
   and d = beta - mean*a, q/k/v = W(a.*x) + (W d + b).  The per-channel a is
   multiplied into the (c,o)-transposed weights (per-partition DVE scalar),
   so projections read the raw bf16 x directly -- no normalized copy of x is
   ever materialized.
 - The k-bias (Wk d + bk) shifts every score of a query by a constant; with
   the deferred softmax normalization below it cancels exactly, so k has NO
   bias at all.
 - Wo is folded into V: ovT = x^T (a.*W_ov^T) with W_ov = Wo@Wv, so the
   attention PV matmul directly produces Wo @ (V A).  The v-bias term
   collapses through softmax rows into bo'' = bo + W_ov d + Wo bv, applied
   with the residual.
 - Softmax without max-subtraction, deferred 1/rowsum (out = r .* (OV A)),
   computed as r = exp(-ln(sum)) on ACT (both funcs live in the single
   loaded natural_log_exp table set; the kernel uses NO other ACT function,
   so exactly one table load ever happens).
 - PV + rowsum run as fp8e4m3 DoubleRow matmuls: exp emits A directly in
   fp8 (scores pre-shifted by -4 so exp(s-4) stays in fp8 range on the
   actual data, max score ~8.2; the shift cancels in the deferred
   normalization), and each matmul contracts TWO 128-wide key tiles at
   2 MACs/cycle -- half the PE streaming and half the LDWEIGHTS swaps.

Scheduling notes:
 - k / ovT production is interleaved into the first attention chunk's
   j-loop (two 512-groups ahead).
 - The PV/rowsum matmuls for pair jp are emitted after the scores matmul
   of pair jp+1 (lag-1 software pipeline).
 - Cross-rep tiles are double-buffered (bufs=2) and the NEXT rep's input
   DMAs + bn_stats are emitted INSIDE the current rep's second attention
   chunk (whose steady state needs no DVE), so in the repeated timing NEFF
   the stats run concurrently with attention instead of serializing at the
   rep boundary.
 - The reciprocal row broadcast [1,512]->[128,512] is a partition-broadcast
   SBUF->SBUF DMA, keeping the epilogue off the PE queue.
"""

import math

import numpy as np

B = 2
C = 128
D = H = W = 16
N_ = 4096
NI = 1024  # queries per core
IC = 512   # i-chunk = one fp32 PSUM bank
JT = 128   # j (key) tile = partition dim
EPS = 1e-5
N_CORES = 8
SHIFT = -4.0  # exp(s + SHIFT); cancels in deferred normalization

# "fp8": PV+rowsum as fp8 DoubleRow; "bf16": plain bf16 attention
MM_MODE = "fp8"

_BUILD_CACHE = {}


def _build(mm_mode, repeat=1):
    from contextlib import ExitStack

    import concourse.mybir as mybir
    import concourse.tile as tile
    from concourse import bacc
    from concourse.bass import ds, ts

    dt = mybir.dt
    f32 = dt.float32

    nc = bacc.Bacc(
        "TRN2", target_bir_lowering=False, debug=False, num_devices=N_CORES
    )

    xb = nc.dram_tensor("xb", (C, N_), dt.bfloat16, kind="ExternalInput").ap()
    xo = nc.dram_tensor("xo", (C, N_), dt.bfloat16, kind="ExternalInput").ap()
    xq = nc.dram_tensor("xq", (C, NI), f32, kind="ExternalInput").ap()
    # wall = [Wq | Wk | Wv | Wo | I] along columns; vecs = [bq bk bv bo gamma beta]
    wall = nc.dram_tensor("wall", (C, 5 * C), f32, kind="ExternalInput").ap()
    vecs = nc.dram_tensor("vecs", (C, 6), f32, kind="ExternalInput").ap()
    out = nc.dram_tensor("out", (C, NI), f32, kind="ExternalOutput").ap()
    dram = {"xb": xb, "xo": xo, "xq": xq, "wall": wall, "vecs": vecs,
            "out": out}

    with tile.TileContext(nc) as tc, ExitStack() as ctx:
        pools = {
            "persist": ctx.enter_context(tc.tile_pool(name="persist", bufs=2)),
            "small": ctx.enter_context(tc.tile_pool(name="small", bufs=2)),
            "work": ctx.enter_context(tc.tile_pool(name="work", bufs=3)),
            "apool": ctx.enter_context(tc.tile_pool(name="apool", bufs=4)),
            # PSUM (8 banks): s pairs 2x2 = 4, h2 1, sum 1, v (prod) 2
            "pss": ctx.enter_context(tc.tile_pool(name="pss", bufs=2, space="PSUM")),
            "psh": ctx.enter_context(tc.tile_pool(name="psh", bufs=1, space="PSUM")),
            "pssum": ctx.enter_context(tc.tile_pool(name="pssum", bufs=1, space="PSUM")),
            "psv": ctx.enter_context(tc.tile_pool(name="psv", bufs=2, space="PSUM")),
        }
        env = (nc, mybir, pools, dram, mm_mode, ts, ds)

        head = _emit_head(env)
        for r in range(repeat):
            nxt = [None]

            def splice(step, nxt=nxt, last=(r == repeat - 1)):
                # called from inside _emit_main during chunk 1
                if last:
                    return
                if step == 0:
                    nxt[0] = _emit_head(env)
                else:
                    _emit_stats(env, nxt[0], step - 1)

            _emit_main(env, head, splice, first=(r == 0))
            head = nxt[0]

    nc.compile()
    return nc


def _emit_head(env):
    """Input DMAs for one rep (double-buffered tiles)."""
    nc, mybir, pools, dram, mm_mode, ts, ds = env
    dt = mybir.dt
    f32 = dt.float32
    bf16 = dt.bfloat16
    persist, small = pools["persist"], pools["small"]

    vecs_sb = small.tile([C, 6], f32, tag="vecs", name="vecs_sb")
    nc.sync.dma_start(out=vecs_sb, in_=dram["vecs"])
    wall_sb = small.tile([C, 5 * C], f32, tag="wall", name="wall_sb")
    nc.sync.dma_start(out=wall_sb, in_=dram["wall"])
    xb_sb = persist.tile([C, N_], bf16, tag="xb", name="xb_sb")
    for h2 in range(2):
        nc.sync.dma_start(
            out=xb_sb[:, ts(h2, 2048)], in_=dram["xb"][:, ts(h2, 2048)]
        )
    xo_sb = persist.tile([C, N_], bf16, tag="xo", name="xo_sb")
    for h2 in range(2):
        nc.sync.dma_start(
            out=xo_sb[:, ts(h2, 2048)], in_=dram["xo"][:, ts(h2, 2048)]
        )
    xq_sb = persist.tile([C, NI], f32, tag="xq", name="xq_sb")
    nc.sync.dma_start(out=xq_sb, in_=dram["xq"])
    st = small.tile([C, 16, 6], f32, tag="st", name="st")
    return {"vecs": vecs_sb, "wall": wall_sb, "xb": xb_sb, "xo": xo_sb,
            "xq": xq_sb, "st": st}


def _emit_stats(env, head, step):
    """Two of the 16 bn_stats chunks (DVE), spliced into chunk-1."""
    nc, mybir, pools, dram, mm_mode, ts, ds = env
    st = head["st"]
    for k in range(2):
        c8 = 2 * step + k
        xt = head["xb"] if c8 < 8 else head["xo"]
        nc.vector.bn_stats(out=st[:, c8, :], in_=xt[:, ds((c8 % 8) * 512, 512)])


def _emit_main(env, head, splice, first):
    nc, mybir, pools, dram, mm_mode, ts, ds = env
    dt = mybir.dt
    f32 = dt.float32
    bf16 = dt.bfloat16
    fp8 = dt.float8e4
    Alu = mybir.AluOpType
    Act = mybir.ActivationFunctionType
    use_fp8 = mm_mode.startswith("fp8")
    a_dt = fp8 if use_fp8 else bf16
    scale = 1.0 / math.sqrt(C)
    NJP = N_ // JT // 2   # 16 j-pairs
    NJG = 8               # production groups of 512
    JG = N_ // NJG
    persist, small, work, apool = (
        pools["persist"], pools["small"], pools["work"], pools["apool"])
    pss, psh, pssum, psv = (
        pools["pss"], pools["psh"], pools["pssum"], pools["psv"])
    out = dram["out"]
    vecs_sb, wall_sb = head["vecs"], head["wall"]
    xb_sb, xo_sb, xq_sb, st = head["xb"], head["xo"], head["xq"], head["st"]

    bq_sb = vecs_sb[:, 0:1]
    bv_sb = vecs_sb[:, 2:3]
    bo_sb = vecs_sb[:, 3:4]
    gamma_sb = vecs_sb[:, 4:5]
    beta_sb = vecs_sb[:, 5:6]
    ident = wall_sb[:, ts(4, C)]

    # ---- constants ----
    ones_row = small.tile([1, C], bf16, tag="ones_row", name="ones_row")
    nc.vector.memset(ones_row, 1.0)
    if use_fp8:
        ones2 = small.tile([C, 2, 16], fp8, tag="ones2", name="ones2")
        nc.vector.memset(ones2, 1.0)
    else:
        ones2 = small.tile([C, 1], bf16, tag="ones2", name="ones2")
        nc.vector.memset(ones2, 1.0)
    z1 = small.tile([C, 1], f32, tag="z1", name="z1")
    nc.vector.memset(z1, 0.0)
    eps_sb = small.tile([C, 1], f32, tag="eps", name="eps_sb")
    nc.vector.memset(eps_sb, EPS)
    shift_sb = small.tile([C, 1], f32, tag="shift", name="shift_sb")
    nc.vector.memset(shift_sb, SHIFT)

    # ---- weight transposes (PE) -> fp32 SBUF copies (DVE) ----
    wT32 = {}
    for i, wname in ((3, "o"), (0, "q"), (1, "k")):
        ps_t = psv.tile([C, C], f32, tag="v", name=f"psT_{wname}")
        nc.tensor.transpose(ps_t, wall_sb[:, ts(i, C)], ident)
        wt = small.tile([C, C], f32, tag=f"wT32_{wname}", name=f"wT32_{wname}")
        nc.vector.tensor_copy(out=wt, in_=ps_t)
        wT32[wname] = wt
    # W_ov^T[c,o'] = sum_o Wv[o,c] * Wo^T[o,o']  (contraction over o)
    ps_ov = psv.tile([C, C], f32, tag="v", name="ps_ov")
    nc.tensor.matmul(ps_ov, wall_sb[:, ts(2, C)], wT32["o"], start=True, stop=True)
    wTov32 = small.tile([C, C], f32, tag="wT32_ov", name="wTov32")
    nc.vector.tensor_copy(out=wTov32, in_=ps_ov)

    # ---- batch stats: bn_stats chunks (spliced into PREVIOUS rep except
    # for the very first), aggregated here ----
    if first:
        for step in range(8):
            _emit_stats(env, head, step)
    mv = small.tile([C, 2], f32, tag="mv", name="mv")
    nc.vector.bn_aggr(out=mv, in_=st)
    mean = mv[:, 0:1]
    var = mv[:, 1:2]
    # invstd = exp(-0.5*ln(var+eps)); a = gamma*invstd; d = beta - mean*a
    lnv = small.tile([C, 1], f32, tag="lnv", name="lnv")
    nc.scalar.activation(lnv, var, Act.Ln, bias=eps_sb, scale=1.0)
    invstd = small.tile([C, 1], f32, tag="invstd", name="invstd")
    nc.scalar.activation(invstd, lnv, Act.Exp, bias=z1, scale=-0.5)
    a_sc = small.tile([C, 1], f32, tag="a_sc", name="a_sc")
    nc.vector.tensor_mul(a_sc, invstd, gamma_sb)
    t0 = small.tile([C, 1], f32, tag="t0", name="t0")
    d_sc = small.tile([C, 1], f32, tag="d_sc", name="d_sc")
    nc.vector.tensor_mul(t0, mean, a_sc)
    nc.vector.tensor_sub(d_sc, beta_sb, t0)

    # ---- scaled bf16 weights ----
    wTq_s = small.tile([C, C], bf16, tag="wTq_s", name="wTq_s")
    nc.vector.tensor_scalar(
        out=wTq_s, in0=wT32["q"], scalar1=a_sc, scalar2=scale,
        op0=Alu.mult, op1=Alu.mult,
    )
    wTk_s = small.tile([C, C], bf16, tag="wTk_s", name="wTk_s")
    nc.vector.tensor_scalar(
        out=wTk_s, in0=wT32["k"], scalar1=a_sc, scalar2=None, op0=Alu.mult
    )
    wTov_s = small.tile([C, C], bf16, tag="wTov_s", name="wTov_s")
    nc.vector.tensor_scalar(
        out=wTov_s, in0=wTov32, scalar1=a_sc, scalar2=None, op0=Alu.mult
    )

    # ---- bias fixups ----
    ps_bq = psv.tile([C, 1], f32, tag="v", name="ps_bq")
    nc.tensor.matmul(ps_bq, wT32["q"], d_sc, start=True, stop=True)
    bq_eff = small.tile([C, 1], f32, tag="bq_eff", name="bq_eff")
    nc.vector.tensor_scalar(
        out=bq_eff, in0=ps_bq, scalar1=bq_sb, scalar2=scale,
        op0=Alu.add, op1=Alu.mult,
    )
    ps_bo = psv.tile([C, 1], f32, tag="v", name="ps_bo")
    nc.tensor.matmul(ps_bo, wTov32, d_sc, start=True, stop=False)
    nc.tensor.matmul(ps_bo, wT32["o"], bv_sb, start=False, stop=True)
    bo_col = small.tile([C, 1], f32, tag="bo_col", name="bo_col")
    nc.vector.tensor_add(bo_col, ps_bo, bo_sb)
    # resid = xq + bo''
    resid = persist.tile([C, NI], f32, tag="resid", name="resid")
    nc.vector.tensor_scalar(
        out=resid, in0=xq_sb, scalar1=bo_col, scalar2=None, op0=Alu.add
    )

    # ---- q[o,i] = wTq''^T x + bq''  (from the fp32 xq slice, cast bf16) ----
    q_sb = persist.tile([C, NI], bf16, tag="q", name="q_sb")
    qx = persist.tile([C, NI], bf16, tag="qx", name="qx")
    nc.vector.tensor_copy(out=qx, in_=xq_sb)
    for i2 in range(NI // 512):
        ps_q = psv.tile([C, 512], f32, tag="v", name="ps_q")
        nc.tensor.matmul(ps_q, wTq_s, qx[:, ts(i2, 512)], start=True, stop=True)
        nc.vector.tensor_scalar(
            out=q_sb[:, ts(i2, 512)], in0=ps_q, scalar1=bq_eff, scalar2=None,
            op0=Alu.add,
        )

    # ---- k / ovT production (interleaved into chunk 0 below) ----
    k_sb = persist.tile([C, N_], bf16, tag="k", name="k_sb")
    ovT = persist.tile([C, N_ // JT, JT], a_dt, tag="ovT", name="ovT")

    def make_k(g):
        ps_k = psv.tile([C, JG], f32, tag="v", name="ps_k")
        nc.tensor.matmul(ps_k, wTk_s, xb_sb[:, ts(g, JG)], start=True, stop=True)
        nc.vector.tensor_copy(out=k_sb[:, ts(g, JG)], in_=ps_k)

    def make_ov(g):
        ps_v = psv.tile([C, JG], f32, tag="v", name="ps_v")
        for t in range(4):
            jt = 4 * g + t
            nc.tensor.matmul(
                ps_v[:, ts(t, JT)], xb_sb[:, ts(jt, JT)], wTov_s,
                start=True, stop=True,
            )
        nc.vector.tensor_copy(out=ovT[:, 4 * g:4 * g + 4, :], in_=ps_v)

    # ---- epilogue pieces: r = exp(-ln(sum)) on ACT right when the sums
    # close; the PE/DVE tail (broadcast, normalize, residual, store) is
    # deferred past the next chunk's first scores so the in-order PE queue
    # never stalls on the ACT reciprocal chain ----
    def emit_epi_head(h2_ps, sum_ps, isl):
        lnr = work.tile([1, IC], f32, tag="lnr", name="lnr")
        nc.scalar.activation(lnr, sum_ps, Act.Ln, bias=z1[0:1, :], scale=1.0)
        r_row = work.tile([1, IC], bf16, tag="r", name="r_row")
        nc.scalar.activation(r_row, lnr, Act.Exp, bias=z1[0:1, :], scale=-1.0)

        def tail():
            rb_ps = psv.tile([C, IC], f32, tag="v", name="rb_ps")
            nc.tensor.matmul(rb_ps, ones_row, r_row, start=True, stop=True)
            rb_sb = work.tile([C, IC], f32, tag="rb", name="rb_sb")
            nc.vector.tensor_copy(out=rb_sb, in_=rb_ps)
            t2 = work.tile([C, IC], f32, tag="t2", name="t2")
            nc.vector.tensor_mul(t2, h2_ps, rb_sb)
            o_sb = work.tile([C, IC], f32, tag="o_sb", name="o_sb")
            nc.vector.tensor_add(o_sb, t2, resid[:, isl])
            nc.sync.dma_start(out=out[:, isl], in_=o_sb)

        return tail

    # ---- attention ----
    epi_tail = None
    for icx in range(NI // IC):
        isl = ds(icx * IC, IC)
        h2_ps = psh.tile([C, IC], f32, tag="h", name="h2_ps")
        sum_ps = pssum.tile([1, IC], f32, tag="sum", name="sum_ps")
        aTs = {}
        if icx == 0:
            make_k(0)
            make_k(1)
            make_ov(0)
            make_ov(1)

        def attend(jp):
            s_ps = pss.tile([C, 2, IC], f32, tag="s", name="s_ps")
            for t in range(2):
                nc.tensor.matmul(
                    s_ps[:, t, :], k_sb[:, ts(2 * jp + t, JT)],
                    q_sb[:, isl], start=True, stop=True,
                )
            aT = apool.tile([C, 2, IC], a_dt, tag="aT", name="aT")
            nc.scalar.activation(
                aT, s_ps, Act.Exp, bias=shift_sb if use_fp8 else z1, scale=1.0
            )
            aTs[jp] = aT

        def accum(jp):
            first_mm, last_mm = jp == 0, jp == NJP - 1
            aT = aTs.pop(jp)
            if use_fp8:
                nc.tensor.matmul(
                    h2_ps, ovT[:, 2 * jp:2 * jp + 2, :], aT,
                    start=first_mm, stop=last_mm,
                    perf_mode=mybir.MatmulPerfMode.DoubleRow,
                )
                nc.tensor.matmul(
                    sum_ps, ones2[:, :, 0:1], aT,
                    start=first_mm, stop=last_mm,
                    perf_mode=mybir.MatmulPerfMode.DoubleRow,
                )
            else:
                for t in range(2):
                    nc.tensor.matmul(
                        h2_ps, ovT[:, 2 * jp + t, :], aT[:, t, :],
                        start=first_mm and t == 0, stop=last_mm and t == 1,
                    )
                    nc.tensor.matmul(
                        sum_ps, ones2, aT[:, t, :],
                        start=first_mm and t == 0, stop=last_mm and t == 1,
                    )

        for jp in range(NJP):
            if icx == 0 and jp % 2 == 0 and jp // 2 + 2 < NJG:
                make_k(jp // 2 + 2)
                make_ov(jp // 2 + 2)
            if icx == 1 and jp <= 8:
                splice(jp)  # next rep: DMAs at jp=0, stats at jp=1..8
            attend(jp)
            if icx == 1 and jp == 1 and epi_tail is not None:
                epi_tail()  # chunk-0 tail; before accum(0) so the h2/sum
                epi_tail = None  # WAR deps see the old readers first
            if jp > 0:
                accum(jp - 1)
        accum(NJP - 1)
        epi_tail = emit_epi_head(h2_ps, sum_ps, isl)
    epi_tail()


def _get_nc(mm_mode=MM_MODE):
    if mm_mode not in _BUILD_CACHE:
        _BUILD_CACHE[mm_mode] = _build(mm_mode)
    return _BUILD_CACHE[mm_mode]


def make_in_maps(inputs, mm_mode=MM_MODE):
    import ml_dtypes

    x = np.ascontiguousarray(
        np.asarray(inputs["inp"], dtype=np.float32).reshape(B, C, N_)
    )
    x_bf = x.astype(ml_dtypes.bfloat16)
    wall = np.ascontiguousarray(np.concatenate(
        [np.asarray(inputs[k], np.float32) for k in ("Wq", "Wk", "Wv", "Wo")]
        + [np.eye(C, dtype=np.float32)],
        axis=1,
    ))
    vecs = np.ascontiguousarray(np.stack(
        [np.asarray(inputs[k], np.float32).reshape(C)
         for k in ("bq", "bk", "bv", "bo", "gamma", "beta")],
        axis=1,
    ))

    in_maps = []
    for core in range(N_CORES):
        b = core // 4
        q0 = (core % 4) * NI
        in_maps.append({
            "xb": np.ascontiguousarray(x_bf[b]),
            "xo": np.ascontiguousarray(x_bf[1 - b]),
            "xq": np.ascontiguousarray(x[b][:, q0:q0 + NI]),
            "wall": wall,
            "vecs": vecs,
        })
    return in_maps


def assemble(results):
    out = np.empty((B, C, N_), dtype=np.float32)
    for core in range(N_CORES):
        b = core // 4
        q0 = (core % 4) * NI
        out[b][:, q0:q0 + NI] = results[core]["out"]
    return out.reshape(B, C, D, H, W)


def run(inputs, mm_mode=MM_MODE, **run_kwargs):
    """Run and return (full_output, BassKernelResults)."""
    from concourse.bass_utils import run_bass_kernel_spmd

    nc = _get_nc(mm_mode)
    in_maps = make_in_maps(inputs, mm_mode)
    res = run_bass_kernel_spmd(
        nc, in_maps, core_ids=list(range(N_CORES)), **run_kwargs
    )
    return assemble(res.results), res


def kernel(**inputs):
    out, _ = run(inputs)
    return out


# revision 21
# speedup vs baseline: 1.4504x; 1.4504x over previous
"""Trainium2 Bass kernel for nn_AttnBlock3d (BatchNorm3d + single-head
self-attention over N=4096 voxels + residual), distributed over 8 NeuronCores.

Sharding: data-parallel over batch (2) x query-quarters (4). Each core
receives its batch's activations (xb), the other batch (xo, stats only),
its query slice (xq, fp32 for the residual), and the weights; it returns
its (C, 1024) output slice. Host assembles the full (B, C, D, H, W) output.

Math notes:
 - BatchNorm folds into the projection weights: with a = gamma*rsqrt(var+eps)
   and d = beta - mean*a, q/k/v = W(a.*x) + (W d + b).  The per-channel a is
   multiplied into the (c,o)-transposed weights (per-partition DVE scalar),
   so projections read the raw bf16 x directly -- no normalized copy of x is
   ever materialized.
 - The k-bias (Wk d + bk) shifts every score of a query by a constant; with
   the deferred softmax normalization below it cancels exactly, so k has NO
   bias at all.
 - Wo is folded into V: ovT = x^T (a.*W_ov^T) with W_ov = Wo@Wv, so the
   attention PV matmul directly produces Wo @ (V A).  The v-bias term
   collapses through softmax rows into bo'' = bo + W_ov d + Wo bv, applied
   with the residual.
 - Softmax without max-subtraction, deferred 1/rowsum (out = r .* (OV A)),
   computed as r = exp(-ln(sum)) on ACT (both funcs live in the single
   loaded natural_log_exp table set; the kernel uses NO other ACT function,
   so exactly one table load ever happens).
 - PV + rowsum run as fp8e4m3 DoubleRow matmuls: exp emits A directly in
   fp8 (scores pre-shifted by -4 so exp(s-4) stays in fp8 range on the
   actual data, max score ~8.2; the shift cancels in the deferred
   normalization), and each matmul contracts TWO 128-wide key tiles at
   2 MACs/cycle -- half the PE streaming and half the LDWEIGHTS swaps.

Scheduling notes:
 - k / ovT production is interleaved into the first attention chunk's
   j-loop (two 512-groups ahead).
 - The PV/rowsum matmuls for pair jp are emitted after the scores matmul
   of pair jp+1 (lag-1 software pipeline).
 - Cross-rep tiles are double-buffered (bufs=2) and the NEXT rep's input
   DMAs + bn_stats are emitted INSIDE the current rep's second attention
   chunk (whose steady state needs no DVE), so in the repeated timing NEFF
   the stats run concurrently with attention instead of serializing at the
   rep boundary.
 - The reciprocal row broadcast [1,512]->[128,512] is a partition-broadcast
   SBUF->SBUF DMA, keeping the epilogue off the PE queue.
"""

import math

import numpy as np

B = 2
C = 128
D = H = W = 16
N_ = 4096
NI = 1024  # queries per core
IC = 512   # i-chunk = one fp32 PSUM bank
JT = 128   # j (key) tile = partition dim
EPS = 1e-5
N_CORES = 8
SHIFT = -4.0  # exp(s + SHIFT); cancels in deferred normalization

# "fp8": PV+rowsum as fp8 DoubleRow; "bf16": plain bf16 attention
MM_MODE = "fp8"

_BUILD_CACHE = {}


def _build(mm_mode, repeat=1):
    from contextlib import ExitStack

    import concourse.mybir as mybir
    import concourse.tile as tile
    from concourse import bacc
    from concourse.bass import ds, ts

    dt = mybir.dt
    f32 = dt.float32

    nc = bacc.Bacc(
        "TRN2", target_bir_lowering=False, debug=False, num_devices=N_CORES
    )

    xb = nc.dram_tensor("xb", (C, N_), dt.bfloat16, kind="ExternalInput").ap()
    xo = nc.dram_tensor("xo", (C, N_), dt.bfloat16, kind="ExternalInput").ap()
    xq = nc.dram_tensor("xq", (C, NI), f32, kind="ExternalInput").ap()
    # wall = [Wq | Wk | Wv | Wo | I] along columns; vecs = [bq bk bv bo gamma beta]
    wall = nc.dram_tensor("wall", (C, 5 * C), f32, kind="ExternalInput").ap()
    vecs = nc.dram_tensor("vecs", (C, 6), f32, kind="ExternalInput").ap()
    out = nc.dram_tensor("out", (C, NI), f32, kind="ExternalOutput").ap()
    dram = {"xb": xb, "xo": xo, "xq": xq, "wall": wall, "vecs": vecs,
            "out": out}

    with tile.TileContext(nc) as tc, ExitStack() as ctx:
        pools = {
            "persist": ctx.enter_context(tc.tile_pool(name="persist", bufs=2)),
            "small": ctx.enter_context(tc.tile_pool(name="small", bufs=2)),
            "work": ctx.enter_context(tc.tile_pool(name="work", bufs=3)),
            "apool": ctx.enter_context(tc.tile_pool(name="apool", bufs=4)),
            # PSUM (8 banks): s pairs 2x2 = 4, h2 1, sum 1, v (prod) 2
            "pss": ctx.enter_context(tc.tile_pool(name="pss", bufs=2, space="PSUM")),
            "psh": ctx.enter_context(tc.tile_pool(name="psh", bufs=1, space="PSUM")),
            "pssum": ctx.enter_context(tc.tile_pool(name="pssum", bufs=1, space="PSUM")),
            "psv": ctx.enter_context(tc.tile_pool(name="psv", bufs=2, space="PSUM")),
        }
        env = (nc, mybir, pools, dram, mm_mode, ts, ds)

        head = _emit_head(env)
        pend = None
        for r in range(repeat):
            nxt = [None]

            def splice(step, nxt=nxt, last=(r == repeat - 1)):
                # called from inside _emit_main during chunk 1
                if last:
                    return
                if step == 0:
                    nxt[0] = _emit_head(env)
                else:
                    _emit_stats(env, nxt[0], step - 1)

            pend = _emit_main(env, head, splice, first=(r == 0), prev_pend=pend)
            head = nxt[0]
        _emit_epilogue(env, pend)

    nc.compile()
    return nc


def _emit_head(env):
    """Input DMAs for one rep (double-buffered tiles)."""
    nc, mybir, pools, dram, mm_mode, ts, ds = env
    dt = mybir.dt
    f32 = dt.float32
    bf16 = dt.bfloat16
    persist, small = pools["persist"], pools["small"]

    vecs_sb = small.tile([C, 6], f32, tag="vecs", name="vecs_sb")
    nc.sync.dma_start(out=vecs_sb, in_=dram["vecs"])
    wall_sb = small.tile([C, 5 * C], f32, tag="wall", name="wall_sb")
    nc.sync.dma_start(out=wall_sb, in_=dram["wall"])
    xb_sb = persist.tile([C, N_], bf16, tag="xb", name="xb_sb")
    for h2 in range(2):
        nc.sync.dma_start(
            out=xb_sb[:, ts(h2, 2048)], in_=dram["xb"][:, ts(h2, 2048)]
        )
    xo_sb = persist.tile([C, N_], bf16, tag="xo", name="xo_sb")
    for h2 in range(2):
        nc.sync.dma_start(
            out=xo_sb[:, ts(h2, 2048)], in_=dram["xo"][:, ts(h2, 2048)]
        )
    xq_sb = persist.tile([C, NI], f32, tag="xq", name="xq_sb")
    nc.sync.dma_start(out=xq_sb, in_=dram["xq"])
    st = small.tile([C, 16, 6], f32, tag="st", name="st")
    return {"vecs": vecs_sb, "wall": wall_sb, "xb": xb_sb, "xo": xo_sb,
            "xq": xq_sb, "st": st}


def _emit_stats(env, head, step):
    """Two of the 16 bn_stats chunks (DVE), spliced into chunk-1."""
    nc, mybir, pools, dram, mm_mode, ts, ds = env
    st = head["st"]
    for k in range(2):
        c8 = 2 * step + k
        xt = head["xb"] if c8 < 8 else head["xo"]
        nc.vector.bn_stats(out=st[:, c8, :], in_=xt[:, ds((c8 % 8) * 512, 512)])


def _emit_epilogue(env, pend):
    """Reciprocal + normalize + residual + store for one rep's two chunks.

    r = exp(-ln(sum)), grouped by ACT function so the two table-set
    switches happen exactly once; the [1,512] row is broadcast down the
    partitions with a rank-1 bf16 matmul.  In the repeated NEFF this is
    emitted near the START of the NEXT rep (ACT and PE are production-bound
    there, DVE is free), so the whole chain pipelines under the next rep's
    prologue instead of serializing at the rep end."""
    nc, mybir, pools, dram, mm_mode, ts, ds = env
    dt = mybir.dt
    f32 = dt.float32
    Act = mybir.ActivationFunctionType
    work = pools["work"]
    psv = pools["psv"]
    out = dram["out"]
    chunks, resid, z1, ones_row = pend
    lns = []
    for ci, (h2_sb, sum_sb, isl) in enumerate(chunks):
        lnr = work.tile([1, IC], f32, tag=f"lnr{ci}", name=f"lnr{ci}")
        nc.scalar.activation(lnr, sum_sb, Act.Ln, bias=z1[0:1, :], scale=1.0)
        lns.append(lnr)
    rows = []
    for ci in range(len(chunks)):
        r_row = work.tile([1, IC], dt.bfloat16, tag=f"r{ci}", name=f"r_row{ci}")
        nc.scalar.activation(r_row, lns[ci], Act.Exp, bias=z1[0:1, :],
                             scale=-1.0)
        rows.append(r_row)
    for ci, (h2_sb, sum_sb, isl) in enumerate(chunks):
        rb_ps = psv.tile([C, IC], f32, tag="v", name=f"rb_ps{ci}")
        nc.tensor.matmul(rb_ps, ones_row, rows[ci], start=True, stop=True)
        rb_sb = work.tile([C, IC], f32, tag=f"rb{ci}", name=f"rb_sb{ci}")
        nc.vector.tensor_copy(out=rb_sb, in_=rb_ps)
        t2 = work.tile([C, IC], f32, tag=f"t2{ci}", name=f"t2{ci}")
        nc.vector.tensor_mul(t2, h2_sb, rb_sb)
        o_sb = work.tile([C, IC], f32, tag=f"o_sb{ci}", name=f"o_sb{ci}")
        nc.vector.tensor_add(o_sb, t2, resid[:, isl])
        nc.sync.dma_start(out=out[:, isl], in_=o_sb)


def _emit_main(env, head, splice, first, prev_pend=None):
    nc, mybir, pools, dram, mm_mode, ts, ds = env
    dt = mybir.dt
    f32 = dt.float32
    bf16 = dt.bfloat16
    fp8 = dt.float8e4
    Alu = mybir.AluOpType
    Act = mybir.ActivationFunctionType
    use_fp8 = mm_mode.startswith("fp8")
    a_dt = fp8 if use_fp8 else bf16
    scale = 1.0 / math.sqrt(C)
    NJP = N_ // JT // 2   # 16 j-pairs
    NJG = 8               # production groups of 512
    JG = N_ // NJG
    persist, small, work, apool = (
        pools["persist"], pools["small"], pools["work"], pools["apool"])
    pss, psh, pssum, psv = (
        pools["pss"], pools["psh"], pools["pssum"], pools["psv"])
    out = dram["out"]
    vecs_sb, wall_sb = head["vecs"], head["wall"]
    xb_sb, xo_sb, xq_sb, st = head["xb"], head["xo"], head["xq"], head["st"]

    bq_sb = vecs_sb[:, 0:1]
    bv_sb = vecs_sb[:, 2:3]
    bo_sb = vecs_sb[:, 3:4]
    gamma_sb = vecs_sb[:, 4:5]
    beta_sb = vecs_sb[:, 5:6]
    ident = wall_sb[:, ts(4, C)]

    # ---- constants ----
    ones_row = small.tile([1, C], bf16, tag="ones_row", name="ones_row")
    nc.vector.memset(ones_row, 1.0)
    if use_fp8:
        ones2 = small.tile([C, 2, 16], fp8, tag="ones2", name="ones2")
        nc.vector.memset(ones2, 1.0)
    else:
        ones2 = small.tile([C, 1], bf16, tag="ones2", name="ones2")
        nc.vector.memset(ones2, 1.0)
    z1 = small.tile([C, 1], f32, tag="z1", name="z1")
    nc.vector.memset(z1, 0.0)
    eps_sb = small.tile([C, 1], f32, tag="eps", name="eps_sb")
    nc.vector.memset(eps_sb, EPS)
    shift_sb = small.tile([C, 1], f32, tag="shift", name="shift_sb")
    nc.vector.memset(shift_sb, SHIFT)

    # ---- weight transposes (PE) -> fp32 SBUF copies (DVE) ----
    wT32 = {}
    for i, wname in ((3, "o"), (0, "q"), (1, "k")):
        ps_t = psv.tile([C, C], f32, tag="v", name=f"psT_{wname}")
        nc.tensor.transpose(ps_t, wall_sb[:, ts(i, C)], ident)
        wt = small.tile([C, C], f32, tag=f"wT32_{wname}", name=f"wT32_{wname}")
        nc.vector.tensor_copy(out=wt, in_=ps_t)
        wT32[wname] = wt
    # W_ov^T[c,o'] = sum_o Wv[o,c] * Wo^T[o,o']  (contraction over o)
    ps_ov = psv.tile([C, C], f32, tag="v", name="ps_ov")
    nc.tensor.matmul(ps_ov, wall_sb[:, ts(2, C)], wT32["o"], start=True, stop=True)
    wTov32 = small.tile([C, C], f32, tag="wT32_ov", name="wTov32")
    nc.vector.tensor_copy(out=wTov32, in_=ps_ov)

    # ---- batch stats: bn_stats chunks (spliced into PREVIOUS rep except
    # for the very first), aggregated here ----
    if first:
        for step in range(8):
            _emit_stats(env, head, step)
    mv = small.tile([C, 2], f32, tag="mv", name="mv")
    nc.vector.bn_aggr(out=mv, in_=st)
    mean = mv[:, 0:1]
    var = mv[:, 1:2]
    # invstd = rsqrt(var+eps) via bit-trick seed + 3 Newton steps, all on
    # DVE [C,1] ops -- avoids Ln/Sqrt ACT table sets entirely (the kernel
    # then uses ONLY Exp, so the ACT table never thrashes)
    u32 = dt.uint32
    vv = small.tile([C, 1], f32, tag="vv", name="vv")
    nc.vector.tensor_scalar(out=vv, in0=var, scalar1=EPS, scalar2=None,
                            op0=Alu.add)
    iu = small.tile([C, 1], u32, tag="iu", name="iu")
    nc.vector.tensor_scalar(out=iu, in0=vv.bitcast(u32), scalar1=1,
                            scalar2=None, op0=Alu.logical_shift_right)
    magic = small.tile([C, 1], u32, tag="magic", name="magic")
    nc.vector.memset(magic, 0x5F3759DF)
    y0u = small.tile([C, 1], u32, tag="y0u", name="y0u")
    nc.vector.tensor_sub(y0u, magic, iu)
    y = y0u.bitcast(f32)
    nt = small.tile([C, 1], f32, tag="nt", name="nt")
    for it in range(3):
        nc.vector.tensor_mul(nt, y, y)
        nc.vector.tensor_mul(nt, nt, vv)
        nc.vector.tensor_scalar(out=nt, in0=nt, scalar1=-0.5, scalar2=1.5,
                                op0=Alu.mult, op1=Alu.add)
        ynew = small.tile([C, 1], f32, tag=f"y{it}", name=f"y{it}")
        nc.vector.tensor_mul(ynew, y, nt)
        y = ynew
    a_sc = small.tile([C, 1], f32, tag="a_sc", name="a_sc")
    nc.vector.tensor_mul(a_sc, y, gamma_sb)
    t0 = small.tile([C, 1], f32, tag="t0", name="t0")
    d_sc = small.tile([C, 1], f32, tag="d_sc", name="d_sc")
    nc.vector.tensor_mul(t0, mean, a_sc)
    nc.vector.tensor_sub(d_sc, beta_sb, t0)

    # ---- scaled bf16 weights ----
    wTq_s = small.tile([C, C], bf16, tag="wTq_s", name="wTq_s")
    nc.vector.tensor_scalar(
        out=wTq_s, in0=wT32["q"], scalar1=a_sc, scalar2=scale,
        op0=Alu.mult, op1=Alu.mult,
    )
    wTk_s = small.tile([C, C], bf16, tag="wTk_s", name="wTk_s")
    nc.vector.tensor_scalar(
        out=wTk_s, in0=wT32["k"], scalar1=a_sc, scalar2=None, op0=Alu.mult
    )
    wTov_s = small.tile([C, C], bf16, tag="wTov_s", name="wTov_s")
    nc.vector.tensor_scalar(
        out=wTov_s, in0=wTov32, scalar1=a_sc, scalar2=None, op0=Alu.mult
    )

    # ---- bias fixups ----
    ps_bq = psv.tile([C, 1], f32, tag="v", name="ps_bq")
    nc.tensor.matmul(ps_bq, wT32["q"], d_sc, start=True, stop=True)
    bq_eff = small.tile([C, 1], f32, tag="bq_eff", name="bq_eff")
    nc.vector.tensor_scalar(
        out=bq_eff, in0=ps_bq, scalar1=bq_sb, scalar2=scale,
        op0=Alu.add, op1=Alu.mult,
    )
    ps_bo = psv.tile([C, 1], f32, tag="v", name="ps_bo")
    nc.tensor.matmul(ps_bo, wTov32, d_sc, start=True, stop=False)
    nc.tensor.matmul(ps_bo, wT32["o"], bv_sb, start=False, stop=True)
    bo_col = small.tile([C, 1], f32, tag="bo_col", name="bo_col")
    nc.vector.tensor_add(bo_col, ps_bo, bo_sb)

    # ---- q[o,i] = wTq''^T x + bq''  (from the fp32 xq slice, cast bf16) ----
    q_sb = persist.tile([C, NI], bf16, tag="q", name="q_sb")
    qx = persist.tile([C, NI], bf16, tag="qx", name="qx")
    nc.vector.tensor_copy(out=qx, in_=xq_sb)
    for i2 in range(NI // 512):
        ps_q = psv.tile([C, 512], f32, tag="v", name="ps_q")
        nc.tensor.matmul(ps_q, wTq_s, qx[:, ts(i2, 512)], start=True, stop=True)
        nc.vector.tensor_scalar(
            out=q_sb[:, ts(i2, 512)], in0=ps_q, scalar1=bq_eff, scalar2=None,
            op0=Alu.add,
        )

    if prev_pend is not None:
        _emit_epilogue(env, prev_pend)

    # ---- k / ovT production (interleaved into chunk 0 below) ----
    k_sb = persist.tile([C, N_], bf16, tag="k", name="k_sb")
    ovT = persist.tile([C, N_ // JT, JT], a_dt, tag="ovT", name="ovT")

    def make_k(g):
        ps_k = psv.tile([C, JG], f32, tag="v", name="ps_k")
        nc.tensor.matmul(ps_k, wTk_s, xb_sb[:, ts(g, JG)], start=True, stop=True)
        nc.vector.tensor_copy(out=k_sb[:, ts(g, JG)], in_=ps_k)

    def make_ov(g):
        ps_v = psv.tile([C, JG], f32, tag="v", name="ps_v")
        for t in range(4):
            jt = 4 * g + t
            nc.tensor.matmul(
                ps_v[:, ts(t, JT)], xb_sb[:, ts(jt, JT)], wTov_s,
                start=True, stop=True,
            )
        nc.vector.tensor_copy(out=ovT[:, 4 * g:4 * g + 4, :], in_=ps_v)

    # resid = xq + bo''  (only needed by the rep-end epilogue; emitted here
    # so it doesn't delay the production-critical DVE chain above)
    resid = persist.tile([C, NI], f32, tag="resid", name="resid")
    nc.vector.tensor_scalar(
        out=resid, in0=xq_sb, scalar1=bo_col, scalar2=None, op0=Alu.add
    )

    # ---- attention; per-chunk results are copied to SBUF when they
    # close (freeing the PSUM banks), and BOTH chunks' reciprocal chains
    # run at the rep end grouped by ACT function (ln ln, exp exp) so the
    # two unavoidable table-set switches happen once per rep, off the
    # attention critical path ----
    pend = []
    for icx in range(NI // IC):
        isl = ds(icx * IC, IC)
        h2_ps = psh.tile([C, IC], f32, tag="h", name="h2_ps")
        sum_ps = pssum.tile([1, IC], f32, tag="sum", name="sum_ps")
        aTs = {}
        if icx == 0:
            make_k(0)
            make_k(1)
            make_ov(0)
            make_ov(1)

        def attend(jp):
            s_ps = pss.tile([C, 2, IC], f32, tag="s", name="s_ps")
            for t in range(2):
                nc.tensor.matmul(
                    s_ps[:, t, :], k_sb[:, ts(2 * jp + t, JT)],
                    q_sb[:, isl], start=True, stop=True,
                )
            aT = apool.tile([C, 2, IC], a_dt, tag="aT", name="aT")
            nc.scalar.activation(
                aT, s_ps, Act.Exp, bias=shift_sb if use_fp8 else z1, scale=1.0
            )
            aTs[jp] = aT

        def accum(jp):
            first_mm, last_mm = jp == 0, jp == NJP - 1
            aT = aTs.pop(jp)
            if use_fp8:
                nc.tensor.matmul(
                    h2_ps, ovT[:, 2 * jp:2 * jp + 2, :], aT,
                    start=first_mm, stop=last_mm,
                    perf_mode=mybir.MatmulPerfMode.DoubleRow,
                )
                nc.tensor.matmul(
                    sum_ps, ones2[:, :, 0:1], aT,
                    start=first_mm, stop=last_mm,
                    perf_mode=mybir.MatmulPerfMode.DoubleRow,
                )
            else:
                for t in range(2):
                    nc.tensor.matmul(
                        h2_ps, ovT[:, 2 * jp + t, :], aT[:, t, :],
                        start=first_mm and t == 0, stop=last_mm and t == 1,
                    )
                    nc.tensor.matmul(
                        sum_ps, ones2, aT[:, t, :],
                        start=first_mm and t == 0, stop=last_mm and t == 1,
                    )

        for jp in range(NJP):
            if icx == 0 and jp % 2 == 0 and jp // 2 + 2 < NJG:
                make_k(jp // 2 + 2)
                make_ov(jp // 2 + 2)
            if icx == 1 and jp <= 8:
                splice(jp)  # next rep: DMAs at jp=0, stats at jp=1..8
            attend(jp)
            if jp > 0:
                accum(jp - 1)
        accum(NJP - 1)
        h2_sb = work.tile([C, IC], f32, tag="h2_sb", name="h2_sb")
        nc.vector.tensor_copy(out=h2_sb, in_=h2_ps)
        sum_sb = work.tile([1, IC], f32, tag="sum_sb", name="sum_sb")
        nc.vector.tensor_copy(out=sum_sb, in_=sum_ps)
        pend.append((h2_sb, sum_sb, isl))

    return (pend, resid, z1, ones_row)


def _get_nc(mm_mode=MM_MODE):
    if mm_mode not in _BUILD_CACHE:
        _BUILD_CACHE[mm_mode] = _build(mm_mode)
    return _BUILD_CACHE[mm_mode]


def make_in_maps(inputs, mm_mode=MM_MODE):
    import ml_dtypes

    x = np.ascontiguousarray(
        np.asarray(inputs["inp"], dtype=np.float32).reshape(B, C, N_)
    )
    x_bf = x.astype(ml_dtypes.bfloat16)
    wall = np.ascontiguousarray(np.concatenate(
        [np.asarray(inputs[k], np.float32) for k in ("Wq", "Wk", "Wv", "Wo")]
        + [np.eye(C, dtype=np.float32)],
        axis=1,
    ))
    vecs = np.ascontiguousarray(np.stack(
        [np.asarray(inputs[k], np.float32).reshape(C)
         for k in ("bq", "bk", "bv", "bo", "gamma", "beta")],
        axis=1,
    ))

    in_maps = []
    for core in range(N_CORES):
        b = core // 4
        q0 = (core % 4) * NI
        in_maps.append({
            "xb": np.ascontiguousarray(x_bf[b]),
            "xo": np.ascontiguousarray(x_bf[1 - b]),
            "xq": np.ascontiguousarray(x[b][:, q0:q0 + NI]),
            "wall": wall,
            "vecs": vecs,
        })
    return in_maps


def assemble(results):
    out = np.empty((B, C, N_), dtype=np.float32)
    for core in range(N_CORES):
        b = core // 4
        q0 = (core % 4) * NI
        out[b][:, q0:q0 + NI] = results[core]["out"]
    return out.reshape(B, C, D, H, W)


def run(inputs, mm_mode=MM_MODE, **run_kwargs):
    """Run and return (full_output, BassKernelResults)."""
    from concourse.bass_utils import run_bass_kernel_spmd

    nc = _get_nc(mm_mode)
    in_maps = make_in_maps(inputs, mm_mode)
    res = run_bass_kernel_spmd(
        nc, in_maps, core_ids=list(range(N_CORES)), **run_kwargs
    )
    return assemble(res.results), res


def kernel(**inputs):
    out, _ = run(inputs)
    return out
